# revision 25
# baseline (speedup 1.0000x reference)
"""DistanceAwareGATv2 on 8 TRN2 NeuronCores (Bass/Tile, SPMD).

Strategy (no collectives):
  - dst ownership: core k owns nodes [k*1250, (k+1)*1250). Within a core,
    nodes are DEGREE-SORTED and assigned one per (tile, partition): node
    rank i -> tile i//128, partition i%128. Each tile handles CH[t] =
    max-degree-in-tile edge slots per partition; a node's edges occupy
    slots (p, 0..deg) on its own partition. Degree sorting makes CH[t]
    decay fast, so total padded slots ~ E/8 + 8%.
  - With dst == partition, the per-dst segment sums are plain free-dim
    reduces (no one-hot scatter matmuls), s2(dst) is a per-partition
    broadcast, and softmax max-subtraction is a per-partition reduce that
    cancels exactly in num/den.
  - x_src materialization: the first NE tiles (default: all 10) compute
    x_proj per edge-slot on the PE from host-staged x[src]^T (pure
    indexing) against [W | W@SW]; any remaining tiles use a gpsimd
    dma_gather from a device-built table (swdge descriptor generation
    costs ~8ns/idx of serial gpsimd time, so NE=NT avoids it entirely
    and shrinks the table build to the 2 batches that supply s2).
    Distance values dm[src,dst] and pad masks are host-packed per-edge
    grids (pure indexing), DMA'd directly. All weight-only folds (W@SW,
    the linearized de-MLP q/c, (j h) column permutation) are host-side.
    All psum->sbuf staging copies run on the scalar engine so the
    in-order vector queue carries only the per-tile chains.

The Bass program is traced per call (shapes specialized to the realized
edge distribution, uniform across cores so one NEFF runs SPMD).
"""
import sys

sys.path.insert(0, "/opt/trn_rl_repo")

import numpy as np

import concourse.bass as bass
import concourse.bacc as bacc
import concourse.mybir as mybir
import concourse.tile as tile
from concourse import library_config
from concourse.bass_utils import run_bass_kernel_spmd

# Problem constants (from the nn module spec).
N, E, IN_CH, H, C, PE_DIM = 10000, 160000, 256, 4, 64, 32
NCORES = 8
NLOC = N // NCORES            # 1250 nodes per core
P = 128
NT = 10                       # (t, p) slots per core = 1280 >= 1250
import os as _os
NE = int(_os.environ.get("KERNEL_NE", "10"))  # tiles on the matmul path (10 = all)
NPAD = 10240                  # padded table rows (80 x 128)
F16 = mybir.dt.float16
F32 = mybir.dt.float32
I16 = mybir.dt.int16


def _pack_idx16(idx: np.ndarray) -> np.ndarray:
    """dma_gather index layout: wrap into 16 partitions, replicate x8."""
    n = idx.shape[0]
    assert n % 16 == 0
    w = idx.reshape(n // 16, 16).T.astype(np.int16)
    return np.tile(w, (8, 1))


def _grid(a: np.ndarray) -> np.ndarray:
    """slot s -> (p, c) = (s % 128, s // 128) grid, [128, CH]."""
    return a.reshape(-1, P).T


def _host_prep(x, edge_index, distance_matrix, W_lin, b_lin, attn,
               de_w1, de_b1, de_w2, de_b2):
    src = np.asarray(edge_index[0]).astype(np.int64)
    dst = np.asarray(edge_index[1]).astype(np.int64)
    x = np.asarray(x, np.float32)
    dm = np.asarray(distance_matrix, np.float32)
    deg = np.bincount(dst, minlength=N)

    # ---- degree-sorted node -> (tile, partition) assignment ------------
    core_sorted = []                      # per core: node id by rank
    rank_of = np.full(N, -1, np.int64)    # rank within owning core
    for k in range(NCORES):
        nodes = np.arange(k * NLOC, (k + 1) * NLOC)
        order = np.argsort(-deg[nodes], kind="stable")
        sn = nodes[order]
        core_sorted.append(sn)
        rank_of[sn] = np.arange(NLOC)

    CH = []
    for t in range(NT):
        mx = 1
        for k in range(NCORES):
            blk = core_sorted[k][t * P:(t + 1) * P]
            if len(blk):
                mx = max(mx, int(deg[blk].max()))
        CH.append(mx)

    # ---- per-edge slot index within its dst node -----------------------
    eo = np.argsort(dst, kind="stable")
    ds = dst[eo]
    first = np.searchsorted(ds, np.arange(N), side="left")
    cidx = np.empty(E, np.int64)
    cidx[eo] = np.arange(E) - first[ds]

    edval = dm[src, dst].astype(np.float16)

    # ---- weight-only folds (host) --------------------------------------
    attn = np.asarray(attn, np.float32)          # [1, H, 2C+PE]
    a1 = attn[0, :, :C]
    a2 = attn[0, :, C:2 * C]
    a3 = attn[0, :, 2 * C:]                      # [H, PE]
    SW = np.zeros((IN_CH, 8), np.float32)        # (h c) col -> (s1|s2) heads
    for h in range(H):
        SW[h * C:(h + 1) * C, h] = a1[h]
        SW[h * C:(h + 1) * C, 4 + h] = a2[h]
    W = np.asarray(W_lin, np.float32)
    # permute x_proj columns to (j h) so every staging copy is layout-free
    perm = np.arange(256).reshape(H, C).T.ravel()    # col j*4+h <- h*64+j
    wfold = np.concatenate([W[:, perm], W @ SW], axis=1)  # [256, 264]

    de_w1 = np.asarray(de_w1, np.float32)        # [1, 16]
    de_b1 = np.asarray(de_b1, np.float32)        # [16]
    de_w2 = np.asarray(de_w2, np.float32)        # [16, 32]
    de_b2 = np.asarray(de_b2, np.float32)        # [32]
    m = de_w2 @ a3.T                             # [16, H]
    cvec = de_b2 @ a3.T                          # [H]
    q = np.maximum(de_w1[0], 0.0) @ m            # [H]
    linear_de = bool((de_b1 == 0).all() and float(dm.min()) >= 0.0)

    cb8 = np.zeros((P, 8), np.float32)
    cb8[:, 4:8] = cvec
    common = {
        "wlin": wfold.astype(np.float16),        # [256, 264]
        "cb8": cb8,                              # [128, 8]
        "qb": np.tile(q.astype(np.float32).reshape(1, H), (P, 1)),
        "w1b": np.tile(de_w1.reshape(1, 16), (P, 1)).astype(np.float32),
        "b1b": np.tile(de_b1.reshape(1, 16), (P, 1)).astype(np.float32),
        "mball": np.tile(m.T.reshape(1, H * 16), (P, 1)).astype(np.float32),
    }

    in_maps = []
    core_of = dst // NLOC
    for k in range(NCORES):
        # permuted table row order: rows 0..1279 = (t,p) slots (rank i),
        # rows 1280.. = all other nodes, tail = zero pad.
        others = np.concatenate(
            [np.arange(0, k * NLOC), np.arange((k + 1) * NLOC, N)])
        rowmap = np.full(NPAD, -1, np.int64)
        rowmap[:NLOC] = core_sorted[k]
        rowmap[NT * P:NT * P + len(others)] = others
        pos_row = np.empty(N, np.int64)
        pos_row[core_sorted[k]] = np.arange(NLOC)
        pos_row[others] = NT * P + np.arange(len(others))

        x_pad = np.zeros((NPAD, IN_CH), np.float32)
        valid = rowmap >= 0
        x_pad[valid] = x[rowmap[valid]]

        ek = np.nonzero(core_of == k)[0]
        es, ec = src[ek], cidx[ek]
        rk = rank_of[dst[ek]]             # 0..1249
        et = rk // P
        ep = rk % P
        eed = edval[ek]

        src_cols, ed_cols, msk_cols, xs_cols = [], [], [], []
        for t in range(NT):
            sel = et == t
            f = ec[sel] * P + ep[sel]     # flat slot = c*128 + p
            n_sl = CH[t] * P
            e_all = np.zeros(n_sl, np.float16)
            m_all = np.zeros(n_sl, np.float16)
            e_all[f] = eed[sel]
            m_all[f] = 1.0
            ed_cols.append(_grid(e_all))
            msk_cols.append(_grid(m_all))
            if t < NE:
                # matmul path: stage raw x rows (transposed) per edge slot
                xsl = np.zeros((n_sl, IN_CH), np.float32)
                xsl[f] = x[es[sel]]
                xs_cols.append(np.ascontiguousarray(xsl.T))
            else:
                s_all = np.zeros(n_sl, np.int64)
                s_all[f] = pos_row[es[sel]]
                src_cols.append(_pack_idx16(s_all))

        mdict = dict(common)
        mdict["xt"] = np.ascontiguousarray(x_pad.T).astype(np.float16)
        mdict["src16"] = (np.concatenate(src_cols, 1) if src_cols
                          else np.zeros((P, 8), np.int16))
        mdict["xst"] = (np.concatenate(xs_cols, 1).astype(np.float16)
                        if xs_cols else np.zeros((IN_CH, P), np.float16))
        mdict["ed16"] = np.concatenate(ed_cols, 1)
        mdict["msk16"] = np.concatenate(msk_cols, 1)
        in_maps.append(mdict)

    meta = {"CH": CH, "linear_de": linear_de}
    return in_maps, meta, core_sorted


def _build(meta):
    CH = meta["CH"]
    SCH = sum(CH)
    GCH = sum(CH[NE:])            # gather-path chunk total
    XC = max(sum(CH[:NE]) * P, P)  # staged matmul-path columns
    nc = bacc.Bacc("TRN2", target_bir_lowering=False)

    # ---------------- I/O ----------------
    t_xt = nc.dram_tensor("xt", [IN_CH, NPAD], F16, kind="ExternalInput")
    t_xst = nc.dram_tensor("xst", [IN_CH, XC], F16, kind="ExternalInput")
    t_w = nc.dram_tensor("wlin", [IN_CH, 264], F16, kind="ExternalInput")
    t_cb8 = nc.dram_tensor("cb8", [P, 8], F32, kind="ExternalInput")
    t_qb = nc.dram_tensor("qb", [P, 4], F32, kind="ExternalInput")
    t_w1b = nc.dram_tensor("w1b", [P, 16], F32, kind="ExternalInput")
    t_b1b = nc.dram_tensor("b1b", [P, 16], F32, kind="ExternalInput")
    t_mball = nc.dram_tensor("mball", [P, H * 16], F32, kind="ExternalInput")
    t_src16 = nc.dram_tensor("src16", [P, max(GCH, 1) * 8], I16,
                             kind="ExternalInput")
    t_ed16 = nc.dram_tensor("ed16", [P, SCH], F16, kind="ExternalInput")
    t_msk16 = nc.dram_tensor("msk16", [P, SCH], F16, kind="ExternalInput")

    t_out = nc.dram_tensor("out", [NT * P, IN_CH], F32, kind="ExternalOutput")

    # internal DRAM table (per-core permuted row order):
    # f16 cols 0:256 = x_proj (j h); f32 cols 128:132 = s1, 132:136 = s2+c
    t_tabx = nc.dram_tensor("tabx", [NPAD, 384], F16)

    with tile.TileContext(nc) as tc:
        with tc.tile_pool(name="const", bufs=1) as const:
            nc.gpsimd.load_library(library_config.mlp)

            # constants / grids (plain DMA loads, no init compute)
            cb8_sb = const.tile([P, 8], F32)
            nc.sync.dma_start(out=cb8_sb[:], in_=t_cb8[:])
            qb = const.tile([P, 4], F32)
            nc.sync.dma_start(out=qb[:], in_=t_qb[:])
            if not meta["linear_de"]:
                w1b = const.tile([P, 16], F32)
                nc.sync.dma_start(out=w1b[:], in_=t_w1b[:])
                b1b = const.tile([P, 16], F32)
                nc.sync.dma_start(out=b1b[:], in_=t_b1b[:])
                mball = const.tile([P, H * 16], F32)
                nc.sync.dma_start(out=mball[:], in_=t_mball[:])
            src16_sb = const.tile([P, max(GCH, 1) * 8], I16)
            nc.sync.dma_start(out=src16_sb[:], in_=t_src16[:])
            ed_sb = const.tile([P, SCH], F16)
            nc.sync.dma_start(out=ed_sb[:], in_=t_ed16[:])
            msk_sb = const.tile([P, SCH], F16)
            nc.sync.dma_start(out=msk_sb[:], in_=t_msk16[:])

            # ---------------- phase 0: x_proj | s1 | s2 table -----------
            # With no gather tiles (NE == NT) the table only supplies s2
            # for the core's own 1280 rows -> 2 batches instead of 10.
            NBATCH = 8
            NBT = 2 if NE >= NT else NPAD // P // NBATCH
            with (
                tc.tile_pool(name="p0", bufs=2) as p0,
                tc.tile_pool(name="xpps", bufs=2 if NE >= NT else 6,
                             space="PSUM") as xppsp,
                tc.tile_pool(name="eps", bufs=6 if NE >= NT else 2,
                             space="PSUM") as epsp,
                tc.tile_pool(name="xstp", bufs=2) as xstp,
                tc.tile_pool(name="fatE", bufs=2) as fatEp,
                tc.tile_pool(name="fatp", bufs=3) as fatp,
                tc.tile_pool(name="ed", bufs=3) as edp,
                tc.tile_pool(name="gp", bufs=2) as gpool,
            ):
                wsb = p0.tile([P, 2, 264], F16, tag="wsb")
                for kb in range(2):
                    nc.sync.dma_start(out=wsb[:, kb, :],
                                      in_=t_w[kb * 128:(kb + 1) * 128, :])
                for bt in range(NBT):
                    xtb = p0.tile([P, 2, NBATCH * P], F16, tag="xtb")
                    for kb in range(2):
                        nc.sync.dma_start(
                            out=xtb[:, kb, :],
                            in_=t_xt[kb * P:(kb + 1) * P,
                                     bt * NBATCH * P:(bt + 1) * NBATCH * P])
                    stagex = p0.tile([P, NBATCH, IN_CH], F16, tag="stagex")
                    stages = p0.tile([P, NBATCH, 8], F32, tag="stages")
                    for a in range(NBATCH):
                        xp_ps = xppsp.tile([P, 264], F32, space="PSUM",
                                           tag="xpps")
                        for kb in range(2):
                            nc.tensor.matmul(
                                out=xp_ps[:],
                                lhsT=xtb[:, kb, a * P:(a + 1) * P],
                                rhs=wsb[:, kb, :],
                                start=(kb == 0), stop=(kb == 1))
                        nc.scalar.copy(out=stagex[:, a, :],
                                       in_=xp_ps[:, 0:256])
                        nc.vector.tensor_tensor(out=stages[:, a, :],
                                                in0=xp_ps[:, 256:264],
                                                in1=cb8_sb[:],
                                                op=mybir.AluOpType.add)
                    r0 = bt * NBATCH * P
                    nc.sync.dma_start(
                        out=t_tabx[r0:r0 + NBATCH * P, 0:256].rearrange(
                            "(a p) c -> p a c", p=P),
                        in_=stagex[:])
                    nc.sync.dma_start(
                        out=t_tabx.bitcast(F32)[r0:r0 + NBATCH * P,
                                                128:136].rearrange(
                            "(a p) c -> p a c", p=P),
                        in_=stages[:])

                # ---------------- phase 1: edge tiles ----------------
                # matmul tiles lead the shared-engine queues (their copies
                # and chains flow right after phase 0); gather tiles have
                # gpsimd to themselves and interleave so fat bufs recycle.
                # matmul tile 0 leads (unblocks the PE/copy pipeline right
                # after phase 0); gather tiles interleave so their chains
                # recycle fat buffers without long stalls.
                torder = []
                ge, ee = NE, 0
                for i in range(NT):
                    if (i % 2 == 0 and ee < NE) or ge >= NT:
                        torder.append(ee); ee += 1
                    else:
                        torder.append(ge); ge += 1
                def assemble(t):
                    ch = CH[t]
                    hc = t < NE

                    if hc:
                        xoff = sum(CH[:t]) * P
                        xsT = xstp.tile([P, 2, ch * P], F16, tag="xst")
                        for kb in range(2):
                            nc.sync.dma_start(
                                out=xsT[:, kb, :],
                                in_=t_xst[kb * P:(kb + 1) * P,
                                          xoff:xoff + ch * P])
                        fat = fatEp.tile([P, ch, 384], F16, tag="fatE")
                        for c in range(ch):
                            ps = epsp.tile([P, 264], F32, space="PSUM",
                                           tag="eps")
                            for kb in range(2):
                                nc.tensor.matmul(
                                    out=ps[:],
                                    lhsT=xsT[:, kb, c * P:(c + 1) * P],
                                    rhs=wsb[:, kb, :],
                                    start=(kb == 0), stop=(kb == 1))
                            nc.scalar.copy(out=fat[:, c, 0:260],
                                           in_=ps[:, 0:260])
                    else:
                        gc0 = sum(CH[NE:t])
                        fat = fatp.tile([P, ch, 384], F16, tag="fat")
                        nc.gpsimd.dma_gather(
                            fat[:], t_tabx[:],
                            src16_sb[:, gc0 * 8:(gc0 + ch) * 8],
                            ch * P, ch * P, 384,
                            single_packet=(ch * P <= 1024))
                    return fat

                def chain(t, fat):
                    ch = CH[t]
                    c0 = sum(CH[:t])
                    hc = t < NE
                    xsrc = fat[:, :, 0:256]
                    # s1: f16 cols (matmul path) or f32 bitcast (gather path)
                    s1 = (fat[:, :, 256:260] if hc
                          else fat[:].bitcast(F32)[:, :, 128:132])

                    # s2(+c) for this tile's 128 dst nodes
                    s2l = edp.tile([P, 4], F32, tag="s2l")
                    nc.sync.dma_start(
                        out=s2l[:],
                        in_=t_tabx.bitcast(F32)[t * P:(t + 1) * P, 132:136])

                    # z = s1 + s2 + a3(de(ed))     [128, ch, 4]
                    z = edp.tile([P, ch, 4], F32, tag="z")
                    s2_b = bass.AP(tensor=s2l.tensor, offset=s2l[:].offset,
                                   ap=[s2l[:].ap[0], [0, ch], [1, 4]])
                    nc.vector.tensor_tensor(out=z[:], in0=s1, in1=s2_b,
                                            op=mybir.AluOpType.add)
                    ed_sl = ed_sb[:, c0:c0 + ch]
                    a3v = edp.tile([P, ch, 4], F32, tag="a3v")
                    if meta["linear_de"]:
                        ed_b = bass.AP(tensor=ed_sb.tensor, offset=ed_sl.offset,
                                       ap=[ed_sl.ap[0], [1, ch], [0, 4]])
                        qb_b = bass.AP(tensor=qb.tensor, offset=qb[:].offset,
                                       ap=[qb[:].ap[0], [0, ch], [1, 4]])
                        nc.vector.tensor_tensor(out=a3v[:], in0=ed_b, in1=qb_b,
                                                op=mybir.AluOpType.mult)
                    else:
                        hid = edp.tile([P, ch, 16], F32, tag="hid")
                        ed_b16 = bass.AP(tensor=ed_sb.tensor,
                                         offset=ed_sl.offset,
                                         ap=[ed_sl.ap[0], [1, ch], [0, 16]])
                        w1_b = bass.AP(tensor=w1b.tensor, offset=w1b[:].offset,
                                       ap=[w1b[:].ap[0], [0, ch], [1, 16]])
                        nc.vector.tensor_tensor(out=hid[:], in0=ed_b16,
                                                in1=w1_b,
                                                op=mybir.AluOpType.mult)
                        b1_b = bass.AP(tensor=b1b.tensor, offset=b1b[:].offset,
                                       ap=[b1b[:].ap[0], [0, ch], [1, 16]])
                        nc.vector.tensor_tensor(out=hid[:], in0=hid[:],
                                                in1=b1_b,
                                                op=mybir.AluOpType.add)
                        nc.scalar.activation(
                            out=hid[:], in_=hid[:],
                            func=mybir.ActivationFunctionType.Relu, scale=1.0)
                        for h in range(H):
                            mb_sl = mball[:, h * 16:(h + 1) * 16]
                            mb_b = bass.AP(tensor=mball.tensor,
                                           offset=mb_sl.offset,
                                           ap=[mb_sl.ap[0], [0, ch], [1, 16]])
                            hm = edp.tile([P, ch, 16], F32, tag="hm")
                            nc.vector.tensor_tensor(out=hm[:], in0=hid[:],
                                                    in1=mb_b,
                                                    op=mybir.AluOpType.mult)
                            nc.vector.tensor_reduce(out=a3v[:, :, h],
                                                    in_=hm[:],
                                                    axis=mybir.AxisListType.X,
                                                    op=mybir.AluOpType.add)
                    nc.vector.tensor_tensor(out=z[:], in0=z[:], in1=a3v[:],
                                            op=mybir.AluOpType.add)
                    # leaky relu(0.2): z = max(z, 0.2 z)
                    nc.vector.scalar_tensor_tensor(
                        out=z[:], in0=z[:], scalar=0.2, in1=z[:],
                        op0=mybir.AluOpType.mult, op1=mybir.AluOpType.max)

                    # per-partition (= per-dst) max; cancels in num/den
                    mx = edp.tile([P, 1], F32, tag="mx")
                    nc.vector.tensor_reduce(out=mx[:], in_=z[:],
                                            axis=mybir.AxisListType.XY,
                                            op=mybir.AluOpType.max)
                    nmx = edp.tile([P, 1], F32, tag="nmx")
                    nc.vector.tensor_scalar_mul(nmx[:], mx[:], -1.0)

                    # alpha = exp(z - mx) * mask    [128, ch, 4] f16
                    am = edp.tile([P, ch, 4], F16, tag="am")
                    nc.scalar.activation(out=am[:], in_=z[:],
                                         func=mybir.ActivationFunctionType.Exp,
                                         bias=nmx[:], scale=1.0)
                    msk_sl = msk_sb[:, c0:c0 + ch]
                    msk_b = bass.AP(tensor=msk_sb.tensor, offset=msk_sl.offset,
                                    ap=[msk_sl.ap[0], [1, ch], [0, 4]])
                    nc.vector.tensor_tensor(out=am[:], in0=am[:], in1=msk_b,
                                            op=mybir.AluOpType.mult)

                    # g = alpha * x_src   ((j h) layout: alpha bcast over j)
                    g = gpool.tile([P, ch, 256], F16, tag="g")
                    al_b = bass.AP(tensor=am.tensor, offset=am[:].offset,
                                   ap=[am[:].ap[0], [4, ch], [0, 64], [1, 4]])
                    nc.vector.tensor_tensor(
                        out=g[:].rearrange("p c (j h) -> p c j h", h=4),
                        in0=xsrc.rearrange("p c (j h) -> p c j h", h=4),
                        in1=al_b, op=mybir.AluOpType.mult)

                    # num = sum_c g: in-place pairwise tree over the chunk
                    # axis (contiguous 256-wide rows). Lands in g[:, 0, :].
                    sz = ch
                    while sz > 1:
                        k = (sz + 1) // 2
                        nc.vector.tensor_tensor(
                            out=g[:, 0:sz - k, :], in0=g[:, 0:sz - k, :],
                            in1=g[:, k:sz, :], op=mybir.AluOpType.add)
                        sz = k
                    den = edp.tile([P, 4], F32, tag="den")
                    nc.vector.tensor_reduce(
                        out=den[:], in_=am[:].rearrange("p c h -> p h c"),
                        axis=mybir.AxisListType.X, op=mybir.AluOpType.add)

                    nc.vector.tensor_scalar_add(den[:], den[:], 1e-30)
                    rec = edp.tile([P, 4], F32, tag="rec")
                    nc.vector.reciprocal(out=rec[:], in_=den[:])
                    o_sb = edp.tile([P, IN_CH], F32, tag="osb")
                    rec_b = bass.AP(tensor=rec.tensor, offset=rec[:].offset,
                                    ap=[rec[:].ap[0], [1, 4], [0, 64]])
                    nc.vector.tensor_tensor(
                        out=o_sb[:].rearrange("p (h j) -> p h j", h=4),
                        in0=g[:, 0, :].rearrange("p (j h) -> p h j", h=4),
                        in1=rec_b, op=mybir.AluOpType.mult)
                    nc.sync.dma_start(out=t_out[t * P:(t + 1) * P, :],
                                      in_=o_sb[:])

                # 1-tile emission skew: tile t+1's matmuls/copies precede
                # tile t's chain in the in-order engine queues, so the exp
                # (scalar) waiting on the vector chain no longer stalls the
                # next tile's psum-draining copies.
                fats = {}
                for i in range(len(torder) + 1):
                    if i < len(torder):
                        fats[torder[i]] = assemble(torder[i])
                    if i >= 1:
                        chain(torder[i - 1], fats.pop(torder[i - 1]))
    nc.compile()
    return nc


LAST_EXEC_NS = None
LAST_TRACE = None


def kernel(**inputs) -> np.ndarray:
    global LAST_EXEC_NS, LAST_TRACE
    import os
    in_maps, meta, core_sorted = _host_prep(
        inputs["x"], inputs["edge_index"], inputs["distance_matrix"],
        inputs["W_lin"], inputs["b_lin"], inputs["attn"],
        inputs["de_w1"], inputs["de_b1"], inputs["de_w2"], inputs["de_b2"])
    nc = _build(meta)
    trace = os.environ.get("KERNEL_TRACE", "0") == "1"
    res = run_bass_kernel_spmd(nc, in_maps, core_ids=list(range(NCORES)),
                               trace=trace)
    if trace:
        LAST_EXEC_NS = res.exec_time_ns
        LAST_TRACE = res.instructions_and_trace
    out = np.empty((N, IN_CH), np.float32)
    for k in range(NCORES):
        out[core_sorted[k]] = res.results[k]["out"][:NLOC]
    return out.astype(np.float32)


# revision 28
# speedup vs baseline: 1.2200x; 1.2200x over previous
"""DistanceAwareGATv2 on 8 TRN2 NeuronCores (Bass/Tile, SPMD).

Strategy (no collectives):
  - dst ownership: core k owns nodes [k*1250, (k+1)*1250). Within a core,
    nodes are DEGREE-SORTED and assigned one per (tile, partition): node
    rank i -> tile i//128, partition i%128. Each tile handles CH[t] =
    max-degree-in-tile edge slots per partition; a node's edges occupy
    slots (p, 0..deg) on its own partition. Degree sorting makes CH[t]
    decay fast, so total padded slots ~ E/8 + 8%.
  - With dst == partition, the per-dst segment sums are plain free-dim
    reduces (no one-hot scatter matmuls), s2(dst) is a per-partition
    broadcast, and softmax max-subtraction is a per-partition reduce that
    cancels exactly in num/den.
  - x_src materialization: the first NE tiles (default: all 10) compute
    x_proj per edge-slot on the PE from host-staged x[src]^T (pure
    indexing) against [W | W@SW]; any remaining tiles use a gpsimd
    dma_gather from a device-built table (swdge descriptor generation
    costs ~8ns/idx of serial gpsimd time, so NE=NT avoids it entirely
    and shrinks the table build to the 2 batches that supply s2).
    Distance values dm[src,dst] and pad masks are host-packed per-edge
    grids (pure indexing), DMA'd directly. All weight-only folds (W@SW,
    the linearized de-MLP q/c, (j h) column permutation) are host-side.
    All psum->sbuf staging copies run on the scalar engine so the
    in-order vector queue carries only the per-tile chains.

The Bass program is traced per call (shapes specialized to the realized
edge distribution, uniform across cores so one NEFF runs SPMD).
"""
import sys

sys.path.insert(0, "/opt/trn_rl_repo")

import numpy as np

import concourse.bass as bass
import concourse.bacc as bacc
import concourse.mybir as mybir
import concourse.tile as tile
from concourse import library_config
from concourse.bass_utils import run_bass_kernel_spmd

# Problem constants (from the nn module spec).
N, E, IN_CH, H, C, PE_DIM = 10000, 160000, 256, 4, 64, 32
NCORES = 8
NLOC = N // NCORES            # 1250 nodes per core
P = 128
NT = 10                       # (t, p) slots per core = 1280 >= 1250
import os as _os
NE = int(_os.environ.get("KERNEL_NE", "10"))  # tiles on the matmul path (10 = all)
NPAD = 10240                  # padded table rows (80 x 128)
F16 = mybir.dt.float16
F32 = mybir.dt.float32
I16 = mybir.dt.int16


def _pack_idx16(idx: np.ndarray) -> np.ndarray:
    """dma_gather index layout: wrap into 16 partitions, replicate x8."""
    n = idx.shape[0]
    assert n % 16 == 0
    w = idx.reshape(n // 16, 16).T.astype(np.int16)
    return np.tile(w, (8, 1))


def _grid(a: np.ndarray) -> np.ndarray:
    """slot s -> (p, c) = (s % 128, s // 128) grid, [128, CH]."""
    return a.reshape(-1, P).T


def _host_prep(x, edge_index, distance_matrix, W_lin, b_lin, attn,
               de_w1, de_b1, de_w2, de_b2):
    src = np.asarray(edge_index[0]).astype(np.int64)
    dst = np.asarray(edge_index[1]).astype(np.int64)
    x = np.asarray(x, np.float32)
    dm = np.asarray(distance_matrix, np.float32)
    deg = np.bincount(dst, minlength=N)

    # ---- degree-sorted node -> (tile, partition) assignment ------------
    core_sorted = []                      # per core: node id by rank
    rank_of = np.full(N, -1, np.int64)    # rank within owning core
    for k in range(NCORES):
        nodes = np.arange(k * NLOC, (k + 1) * NLOC)
        order = np.argsort(-deg[nodes], kind="stable")
        sn = nodes[order]
        core_sorted.append(sn)
        rank_of[sn] = np.arange(NLOC)

    CH = []
    for t in range(NT):
        mx = 1
        for k in range(NCORES):
            blk = core_sorted[k][t * P:(t + 1) * P]
            if len(blk):
                mx = max(mx, int(deg[blk].max()))
        CH.append(mx)

    # ---- per-edge slot index within its dst node -----------------------
    eo = np.argsort(dst, kind="stable")
    ds = dst[eo]
    first = np.searchsorted(ds, np.arange(N), side="left")
    cidx = np.empty(E, np.int64)
    cidx[eo] = np.arange(E) - first[ds]

    edval = dm[src, dst].astype(np.float16)

    # ---- weight-only folds (host) --------------------------------------
    attn = np.asarray(attn, np.float32)          # [1, H, 2C+PE]
    a1 = attn[0, :, :C]
    a2 = attn[0, :, C:2 * C]
    a3 = attn[0, :, 2 * C:]                      # [H, PE]
    SW = np.zeros((IN_CH, 8), np.float32)        # (h c) col -> (s1|s2) heads
    for h in range(H):
        SW[h * C:(h + 1) * C, h] = a1[h]
        SW[h * C:(h + 1) * C, 4 + h] = a2[h]
    W = np.asarray(W_lin, np.float32)
    # permute x_proj columns to (j h) so every staging copy is layout-free
    perm = np.arange(256).reshape(H, C).T.ravel()    # col j*4+h <- h*64+j
    wfold = np.concatenate([W[:, perm], W @ SW], axis=1)  # [256, 264]

    de_w1 = np.asarray(de_w1, np.float32)        # [1, 16]
    de_b1 = np.asarray(de_b1, np.float32)        # [16]
    de_w2 = np.asarray(de_w2, np.float32)        # [16, 32]
    de_b2 = np.asarray(de_b2, np.float32)        # [32]
    m = de_w2 @ a3.T                             # [16, H]
    cvec = de_b2 @ a3.T                          # [H]
    q = np.maximum(de_w1[0], 0.0) @ m            # [H]
    linear_de = bool((de_b1 == 0).all() and float(dm.min()) >= 0.0)

    cb8 = np.zeros((P, 8), np.float32)
    cb8[:, 4:8] = cvec
    common = {
        "wlin": wfold.astype(np.float16),        # [256, 264]
        "cb8": cb8,                              # [128, 8]
        "qb": np.tile(q.astype(np.float32).reshape(1, H), (P, 1)),
        "w1b": np.tile(de_w1.reshape(1, 16), (P, 1)).astype(np.float32),
        "b1b": np.tile(de_b1.reshape(1, 16), (P, 1)).astype(np.float32),
        "mball": np.tile(m.T.reshape(1, H * 16), (P, 1)).astype(np.float32),
    }

    in_maps = []
    core_of = dst // NLOC
    for k in range(NCORES):
        # permuted table row order: rows 0..1279 = (t,p) slots (rank i),
        # rows 1280.. = all other nodes, tail = zero pad.
        others = np.concatenate(
            [np.arange(0, k * NLOC), np.arange((k + 1) * NLOC, N)])
        rowmap = np.full(NPAD, -1, np.int64)
        rowmap[:NLOC] = core_sorted[k]
        rowmap[NT * P:NT * P + len(others)] = others
        pos_row = np.empty(N, np.int64)
        pos_row[core_sorted[k]] = np.arange(NLOC)
        pos_row[others] = NT * P + np.arange(len(others))

        x_pad = np.zeros((NPAD, IN_CH), np.float32)
        valid = rowmap >= 0
        x_pad[valid] = x[rowmap[valid]]

        ek = np.nonzero(core_of == k)[0]
        es, ec = src[ek], cidx[ek]
        rk = rank_of[dst[ek]]             # 0..1249
        et = rk // P
        ep = rk % P
        eed = edval[ek]

        src_cols, ed_cols, msk_cols, xs_cols = [], [], [], []
        for t in range(NT):
            sel = et == t
            f = ec[sel] * P + ep[sel]     # flat slot = c*128 + p
            n_sl = CH[t] * P
            e_all = np.zeros(n_sl, np.float16)
            m_all = np.zeros(n_sl, np.float16)
            e_all[f] = eed[sel]
            m_all[f] = 1.0
            ed_cols.append(_grid(e_all))
            msk_cols.append(_grid(m_all))
            if t < NE:
                # matmul path: stage raw x rows (transposed) per edge slot
                xsl = np.zeros((n_sl, IN_CH), np.float32)
                xsl[f] = x[es[sel]]
                xs_cols.append(np.ascontiguousarray(xsl.T))
            else:
                s_all = np.zeros(n_sl, np.int64)
                s_all[f] = pos_row[es[sel]]
                src_cols.append(_pack_idx16(s_all))

        mdict = dict(common)
        mdict["xt"] = np.ascontiguousarray(x_pad.T).astype(np.float16)
        mdict["src16"] = (np.concatenate(src_cols, 1) if src_cols
                          else np.zeros((P, 8), np.int16))
        mdict["xst"] = (np.concatenate(xs_cols, 1).astype(np.float16)
                        if xs_cols else np.zeros((IN_CH, P), np.float16))
        mdict["ed16"] = np.concatenate(ed_cols, 1)
        mdict["msk16"] = np.concatenate(msk_cols, 1)
        in_maps.append(mdict)

    meta = {"CH": CH, "linear_de": linear_de}
    return in_maps, meta, core_sorted


def _build(meta):
    CH = meta["CH"]
    SCH = sum(CH)
    GCH = sum(CH[NE:])            # gather-path chunk total
    XC = max(sum(CH[:NE]) * P, P)  # staged matmul-path columns
    nc = bacc.Bacc("TRN2", target_bir_lowering=False)

    # ---------------- I/O ----------------
    t_xt = nc.dram_tensor("xt", [IN_CH, NPAD], F16, kind="ExternalInput")
    t_xst = nc.dram_tensor("xst", [IN_CH, XC], F16, kind="ExternalInput")
    t_w = nc.dram_tensor("wlin", [IN_CH, 264], F16, kind="ExternalInput")
    t_cb8 = nc.dram_tensor("cb8", [P, 8], F32, kind="ExternalInput")
    t_qb = nc.dram_tensor("qb", [P, 4], F32, kind="ExternalInput")
    t_w1b = nc.dram_tensor("w1b", [P, 16], F32, kind="ExternalInput")
    t_b1b = nc.dram_tensor("b1b", [P, 16], F32, kind="ExternalInput")
    t_mball = nc.dram_tensor("mball", [P, H * 16], F32, kind="ExternalInput")
    t_src16 = nc.dram_tensor("src16", [P, max(GCH, 1) * 8], I16,
                             kind="ExternalInput")
    t_ed16 = nc.dram_tensor("ed16", [P, SCH], F16, kind="ExternalInput")
    t_msk16 = nc.dram_tensor("msk16", [P, SCH], F16, kind="ExternalInput")

    t_out = nc.dram_tensor("out", [NT * P, IN_CH], F32, kind="ExternalOutput")

    # internal DRAM table (per-core permuted row order):
    # f16 cols 0:256 = x_proj (j h); f32 cols 128:132 = s1, 132:136 = s2+c
    t_tabx = nc.dram_tensor("tabx", [NPAD, 384], F16)

    with tile.TileContext(nc) as tc:
        with tc.tile_pool(name="const", bufs=1) as const:
            nc.gpsimd.load_library(library_config.mlp)

            # constants / grids (plain DMA loads, no init compute)
            cb8_sb = const.tile([P, 8], F32)
            nc.sync.dma_start(out=cb8_sb[:], in_=t_cb8[:])
            qb = const.tile([P, 4], F32)
            nc.sync.dma_start(out=qb[:], in_=t_qb[:])
            if not meta["linear_de"]:
                w1b = const.tile([P, 16], F32)
                nc.sync.dma_start(out=w1b[:], in_=t_w1b[:])
                b1b = const.tile([P, 16], F32)
                nc.sync.dma_start(out=b1b[:], in_=t_b1b[:])
                mball = const.tile([P, H * 16], F32)
                nc.sync.dma_start(out=mball[:], in_=t_mball[:])
            src16_sb = const.tile([P, max(GCH, 1) * 8], I16)
            nc.sync.dma_start(out=src16_sb[:], in_=t_src16[:])
            ed_sb = const.tile([P, SCH], F16)
            nc.sync.dma_start(out=ed_sb[:], in_=t_ed16[:])
            msk_sb = const.tile([P, SCH], F16)
            nc.sync.dma_start(out=msk_sb[:], in_=t_msk16[:])

            # ---------------- phase 0: x_proj | s1 | s2 table -----------
            # With no gather tiles (NE == NT) the table only supplies s2
            # for the core's own 1280 rows -> 2 batches instead of 10.
            NBATCH = 8
            NBT = 2 if NE >= NT else NPAD // P // NBATCH
            with (
                tc.tile_pool(name="p0", bufs=2) as p0,
                tc.tile_pool(name="xpps", bufs=2 if NE >= NT else 6,
                             space="PSUM") as xppsp,
                tc.tile_pool(name="eps", bufs=3 if NE >= NT else 2,
                             space="PSUM") as epsp,
                tc.tile_pool(name="xstp", bufs=2) as xstp,
                tc.tile_pool(name="fatE", bufs=2) as fatEp,
                tc.tile_pool(name="fatp", bufs=3) as fatp,
                tc.tile_pool(name="ed", bufs=3) as edp,
                tc.tile_pool(name="gp", bufs=2) as gpool,
            ):
                wsb = p0.tile([P, 2, 264], F16, tag="wsb")
                for kb in range(2):
                    nc.sync.dma_start(out=wsb[:, kb, :],
                                      in_=t_w[kb * 128:(kb + 1) * 128, :])
                for bt in range(NBT):
                    xtb = p0.tile([P, 2, NBATCH * P], F16, tag="xtb")
                    for kb in range(2):
                        nc.sync.dma_start(
                            out=xtb[:, kb, :],
                            in_=t_xt[kb * P:(kb + 1) * P,
                                     bt * NBATCH * P:(bt + 1) * NBATCH * P])
                    stagex = p0.tile([P, NBATCH, IN_CH], F16, tag="stagex")
                    stages = p0.tile([P, NBATCH, 8], F32, tag="stages")
                    for a in range(NBATCH):
                        xp_ps = xppsp.tile([P, 264], F32, space="PSUM",
                                           tag="xpps")
                        for kb in range(2):
                            nc.tensor.matmul(
                                out=xp_ps[:],
                                lhsT=xtb[:, kb, a * P:(a + 1) * P],
                                rhs=wsb[:, kb, :],
                                start=(kb == 0), stop=(kb == 1))
                        nc.scalar.copy(out=stagex[:, a, :],
                                       in_=xp_ps[:, 0:256])
                        nc.vector.tensor_tensor(out=stages[:, a, :],
                                                in0=xp_ps[:, 256:264],
                                                in1=cb8_sb[:],
                                                op=mybir.AluOpType.add)
                    r0 = bt * NBATCH * P
                    nc.sync.dma_start(
                        out=t_tabx[r0:r0 + NBATCH * P, 0:256].rearrange(
                            "(a p) c -> p a c", p=P),
                        in_=stagex[:])
                    nc.sync.dma_start(
                        out=t_tabx.bitcast(F32)[r0:r0 + NBATCH * P,
                                                128:136].rearrange(
                            "(a p) c -> p a c", p=P),
                        in_=stages[:])

                # ---------------- phase 1: edge tiles ----------------
                # matmul tiles lead the shared-engine queues (their copies
                # and chains flow right after phase 0); gather tiles have
                # gpsimd to themselves and interleave so fat bufs recycle.
                # matmul tile 0 leads (unblocks the PE/copy pipeline right
                # after phase 0); gather tiles interleave so their chains
                # recycle fat buffers without long stalls.
                torder = []
                ge, ee = NE, 0
                for i in range(NT):
                    if (i % 2 == 0 and ee < NE) or ge >= NT:
                        torder.append(ee); ee += 1
                    else:
                        torder.append(ge); ge += 1
                def assemble(t):
                    ch = CH[t]
                    hc = t < NE

                    if hc:
                        xoff = sum(CH[:t]) * P
                        xsT = xstp.tile([P, 2, ch * P], F16, tag="xst")
                        for kb in range(2):
                            nc.sync.dma_start(
                                out=xsT[:, kb, :],
                                in_=t_xst[kb * P:(kb + 1) * P,
                                          xoff:xoff + ch * P])
                        fat = fatEp.tile([P, ch, 384], F16, tag="fatE")
                        # chunk pairs share a 2-bank psum tile (each matmul
                        # out is bank-aligned) -> one staging copy per pair
                        for c in range(0, ch - 1, 2):
                            ps2 = epsp.tile([P, 2, 512], F32, space="PSUM",
                                            tag="eps2")
                            for i in range(2):
                                for kb in range(2):
                                    nc.tensor.matmul(
                                        out=ps2[:, i, 0:264],
                                        lhsT=xsT[:, kb,
                                                 (c + i) * P:(c + i + 1) * P],
                                        rhs=wsb[:, kb, :],
                                        start=(kb == 0), stop=(kb == 1))
                            nc.scalar.copy(out=fat[:, c:c + 2, 0:260],
                                           in_=ps2[:, :, 0:260])
                        if ch % 2:
                            c = ch - 1
                            ps2 = epsp.tile([P, 2, 512], F32, space="PSUM",
                                            tag="eps2")
                            for kb in range(2):
                                nc.tensor.matmul(
                                    out=ps2[:, 0, 0:264],
                                    lhsT=xsT[:, kb, c * P:(c + 1) * P],
                                    rhs=wsb[:, kb, :],
                                    start=(kb == 0), stop=(kb == 1))
                            nc.scalar.copy(out=fat[:, c, 0:260],
                                           in_=ps2[:, 0, 0:260])
                    else:
                        gc0 = sum(CH[NE:t])
                        fat = fatp.tile([P, ch, 384], F16, tag="fat")
                        nc.gpsimd.dma_gather(
                            fat[:], t_tabx[:],
                            src16_sb[:, gc0 * 8:(gc0 + ch) * 8],
                            ch * P, ch * P, 384,
                            single_packet=(ch * P <= 1024))
                    return fat

                def chain(t, fat):
                    ch = CH[t]
                    c0 = sum(CH[:t])
                    hc = t < NE
                    xsrc = fat[:, :, 0:256]
                    # s1: f16 cols (matmul path) or f32 bitcast (gather path)
                    s1 = (fat[:, :, 256:260] if hc
                          else fat[:].bitcast(F32)[:, :, 128:132])

                    # s2(+c) for this tile's 128 dst nodes
                    s2l = edp.tile([P, 4], F32, tag="s2l")
                    nc.sync.dma_start(
                        out=s2l[:],
                        in_=t_tabx.bitcast(F32)[t * P:(t + 1) * P, 132:136])

                    # z = s1 + s2 + a3(de(ed))     [128, ch, 4]
                    z = edp.tile([P, ch, 4], F32, tag="z")
                    s2_b = bass.AP(tensor=s2l.tensor, offset=s2l[:].offset,
                                   ap=[s2l[:].ap[0], [0, ch], [1, 4]])
                    nc.vector.tensor_tensor(out=z[:], in0=s1, in1=s2_b,
                                            op=mybir.AluOpType.add)
                    ed_sl = ed_sb[:, c0:c0 + ch]
                    a3v = edp.tile([P, ch, 4], F32, tag="a3v")
                    if meta["linear_de"]:
                        ed_b = bass.AP(tensor=ed_sb.tensor, offset=ed_sl.offset,
                                       ap=[ed_sl.ap[0], [1, ch], [0, 4]])
                        qb_b = bass.AP(tensor=qb.tensor, offset=qb[:].offset,
                                       ap=[qb[:].ap[0], [0, ch], [1, 4]])
                        nc.vector.tensor_tensor(out=a3v[:], in0=ed_b, in1=qb_b,
                                                op=mybir.AluOpType.mult)
                    else:
                        hid = edp.tile([P, ch, 16], F32, tag="hid")
                        ed_b16 = bass.AP(tensor=ed_sb.tensor,
                                         offset=ed_sl.offset,
                                         ap=[ed_sl.ap[0], [1, ch], [0, 16]])
                        w1_b = bass.AP(tensor=w1b.tensor, offset=w1b[:].offset,
                                       ap=[w1b[:].ap[0], [0, ch], [1, 16]])
                        nc.vector.tensor_tensor(out=hid[:], in0=ed_b16,
                                                in1=w1_b,
                                                op=mybir.AluOpType.mult)
                        b1_b = bass.AP(tensor=b1b.tensor, offset=b1b[:].offset,
                                       ap=[b1b[:].ap[0], [0, ch], [1, 16]])
                        nc.vector.tensor_tensor(out=hid[:], in0=hid[:],
                                                in1=b1_b,
                                                op=mybir.AluOpType.add)
                        nc.scalar.activation(
                            out=hid[:], in_=hid[:],
                            func=mybir.ActivationFunctionType.Relu, scale=1.0)
                        for h in range(H):
                            mb_sl = mball[:, h * 16:(h + 1) * 16]
                            mb_b = bass.AP(tensor=mball.tensor,
                                           offset=mb_sl.offset,
                                           ap=[mb_sl.ap[0], [0, ch], [1, 16]])
                            hm = edp.tile([P, ch, 16], F32, tag="hm")
                            nc.vector.tensor_tensor(out=hm[:], in0=hid[:],
                                                    in1=mb_b,
                                                    op=mybir.AluOpType.mult)
                            nc.vector.tensor_reduce(out=a3v[:, :, h],
                                                    in_=hm[:],
                                                    axis=mybir.AxisListType.X,
                                                    op=mybir.AluOpType.add)
                    nc.vector.tensor_tensor(out=z[:], in0=z[:], in1=a3v[:],
                                            op=mybir.AluOpType.add)
                    # leaky relu(0.2): z = max(z, 0.2 z)
                    nc.vector.scalar_tensor_tensor(
                        out=z[:], in0=z[:], scalar=0.2, in1=z[:],
                        op0=mybir.AluOpType.mult, op1=mybir.AluOpType.max)

                    # per-partition (= per-dst) max; cancels in num/den
                    mx = edp.tile([P, 1], F32, tag="mx")
                    nc.vector.tensor_reduce(out=mx[:], in_=z[:],
                                            axis=mybir.AxisListType.XY,
                                            op=mybir.AluOpType.max)
                    nmx = edp.tile([P, 1], F32, tag="nmx")
                    nc.vector.tensor_scalar_mul(nmx[:], mx[:], -1.0)

                    # alpha = exp(z - mx) * mask    [128, ch, 4] f16
                    am = edp.tile([P, ch, 4], F16, tag="am")
                    nc.scalar.activation(out=am[:], in_=z[:],
                                         func=mybir.ActivationFunctionType.Exp,
                                         bias=nmx[:], scale=1.0)
                    msk_sl = msk_sb[:, c0:c0 + ch]
                    msk_b = bass.AP(tensor=msk_sb.tensor, offset=msk_sl.offset,
                                    ap=[msk_sl.ap[0], [1, ch], [0, 4]])
                    nc.vector.tensor_tensor(out=am[:], in0=am[:], in1=msk_b,
                                            op=mybir.AluOpType.mult)

                    # g = alpha * x_src   ((j h) layout: alpha bcast over j)
                    g = gpool.tile([P, ch, 256], F16, tag="g")
                    al_b = bass.AP(tensor=am.tensor, offset=am[:].offset,
                                   ap=[am[:].ap[0], [4, ch], [0, 64], [1, 4]])
                    nc.vector.tensor_tensor(
                        out=g[:].rearrange("p c (j h) -> p c j h", h=4),
                        in0=xsrc.rearrange("p c (j h) -> p c j h", h=4),
                        in1=al_b, op=mybir.AluOpType.mult)

                    # num = sum_c g: in-place pairwise tree over the chunk
                    # axis (contiguous 256-wide rows). Lands in g[:, 0, :].
                    sz = ch
                    while sz > 1:
                        k = (sz + 1) // 2
                        nc.vector.tensor_tensor(
                            out=g[:, 0:sz - k, :], in0=g[:, 0:sz - k, :],
                            in1=g[:, k:sz, :], op=mybir.AluOpType.add)
                        sz = k
                    den = edp.tile([P, 4], F32, tag="den")
                    nc.vector.tensor_reduce(
                        out=den[:], in_=am[:].rearrange("p c h -> p h c"),
                        axis=mybir.AxisListType.X, op=mybir.AluOpType.add)

                    nc.vector.tensor_scalar_add(den[:], den[:], 1e-30)
                    rec = edp.tile([P, 4], F32, tag="rec")
                    nc.vector.reciprocal(out=rec[:], in_=den[:])
                    o_sb = edp.tile([P, IN_CH], F32, tag="osb")
                    rec_b = bass.AP(tensor=rec.tensor, offset=rec[:].offset,
                                    ap=[rec[:].ap[0], [1, 4], [0, 64]])
                    nc.vector.tensor_tensor(
                        out=o_sb[:].rearrange("p (h j) -> p h j", h=4),
                        in0=g[:, 0, :].rearrange("p (j h) -> p h j", h=4),
                        in1=rec_b, op=mybir.AluOpType.mult)
                    nc.sync.dma_start(out=t_out[t * P:(t + 1) * P, :],
                                      in_=o_sb[:])

                for t in torder:
                    chain(t, assemble(t))
    nc.compile()
    return nc


LAST_EXEC_NS = None
LAST_TRACE = None


def kernel(**inputs) -> np.ndarray:
    global LAST_EXEC_NS, LAST_TRACE
    import os
    in_maps, meta, core_sorted = _host_prep(
        inputs["x"], inputs["edge_index"], inputs["distance_matrix"],
        inputs["W_lin"], inputs["b_lin"], inputs["attn"],
        inputs["de_w1"], inputs["de_b1"], inputs["de_w2"], inputs["de_b2"])
    nc = _build(meta)
    trace = os.environ.get("KERNEL_TRACE", "0") == "1"
    res = run_bass_kernel_spmd(nc, in_maps, core_ids=list(range(NCORES)),
                               trace=trace)
    if trace:
        LAST_EXEC_NS = res.exec_time_ns
        LAST_TRACE = res.instructions_and_trace
    out = np.empty((N, IN_CH), np.float32)
    for k in range(NCORES):
        out[core_sorted[k]] = res.results[k]["out"][:NLOC]
    return out.astype(np.float32)
